# revision 39
# baseline (speedup 1.0000x reference)
"""Multi-head attention block (B=4, S=2048, D=1024, H=16) on 8 trn2 NeuronCores.

Sharding: core c -> (batch b = c//2, head-half hh = c%2). Each core computes
8 heads of one batch element: Q/K/V projections for its 512 output dims,
attention, and a partial output projection (row-shard of W_o). Host sums the
two partials per batch and adds the bias terms (b_o + b_v @ W_o; the V bias
passes through attention unchanged because softmax weights sum to 1).

Device dataflow (all matmuls bf16 with fp32 PSUM accumulation):
  Q^T = Wq^T @ xq^T          [512, 2048]   (lhsT=Wq, rhs=xq^T)
  K^T = Wk^T @ xk^T          [512, 2048]
  V   = xv^T.T @ Wv          [2048, 512]
  S^T_h = K_h Q_h^T          [S_k, S_q]    two heads packed per PE pass (K=64 row tiling)
  E = exp(S^T / 8)           ScalarE, fused scale
  O^T_h = V_h^T @ E          [64, S_q]     two heads packed per PE pass (M=64 col tiling)
  den_h = ones^T @ E         [1, S_q]      M=1 col-tiled matmuls
  O^T  /= den (recip + partition-broadcast DMA + multiply)
  out  = O^T.T @ Wo          [2048, 1024]  partial, summed on host
"""

import sys

if "/opt/trn_rl_repo" not in sys.path:
    sys.path.insert(0, "/opt/trn_rl_repo")

import numpy as np
import ml_dtypes

import concourse.bass as bass
import concourse.tile as tile
from concourse import bacc, mybir

BF16 = ml_dtypes.bfloat16

B, S, D = 4, 2048, 1024
H, DK = 16, 64
HPC = 8          # heads per core
DPC = HPC * DK   # 512 output dims per core
P = 128
N_CHUNK = 512    # S_q chunk
N_CHUNKS = S // N_CHUNK          # 4
KT = S // P                      # 16 S_k tiles
KO = D // P                      # 8 contraction tiles for projections
PO = DPC // P                    # 4 partition tiles of the 512-dim axis


def _build_program(with_mask: bool, has_bias: bool = True, for_sim: bool = False):
    f32 = mybir.dt.float32
    bf16 = mybir.dt.bfloat16

    # Tile kernels must be built on Bacc: its compile() legalizes multi-wait
    # sync_info (event semaphores) that walrus codegen cannot ingest raw.
    if for_sim:
        nc = bacc.Bacc(None, target_bir_lowering=False, debug=True)
    else:
        nc = bacc.Bacc(None)

    xqT = nc.dram_tensor("xqT", [D, S], bf16, kind="ExternalInput")
    xkT = nc.dram_tensor("xkT", [D, S], bf16, kind="ExternalInput")
    xvT = nc.dram_tensor("xvT", [D, S], bf16, kind="ExternalInput")
    wq = nc.dram_tensor("wq", [D, DPC], bf16, kind="ExternalInput")
    wk = nc.dram_tensor("wk", [D, DPC], bf16, kind="ExternalInput")
    wv = nc.dram_tensor("wv", [D, DPC], bf16, kind="ExternalInput")
    wo = nc.dram_tensor("wo", [DPC, D], bf16, kind="ExternalInput")
    bq = nc.dram_tensor("bq", [DPC], bf16, kind="ExternalInput")
    bk = nc.dram_tensor("bk", [DPC], bf16, kind="ExternalInput")
    bv = nc.dram_tensor("bv", [DPC], bf16, kind="ExternalInput")
    if with_mask:
        mbias = nc.dram_tensor("mbias", [S, S], bf16, kind="ExternalInput")
    out = nc.dram_tensor("out", [S, D], f32, kind="ExternalOutput")

    with tile.TileContext(nc) as tc:
        with (
            tc.tile_pool(name="persist", bufs=1) as persist,
            tc.tile_pool(name="xstage", bufs=3) as xstage,
            tc.tile_pool(name="work", bufs=3) as work,
            tc.tile_pool(name="psum", bufs=1, space="PSUM") as psum,
            tc.tile_pool(name="spsum_pool", bufs=2, space="PSUM") as spsum_pool,
            tc.tile_pool(name="mm512_pool", bufs=2, space="PSUM") as mm512_pool,
            tc.tile_pool(name="dram", bufs=2, space="DRAM") as dram,
        ):
            # ---- resident weights / biases / persistent activations
            # small constants first, then weight loads split per-ko so they
            # spread across DMA queues; K and Q first (first compute), then V,
            # W_o last (needed latest)
            def dma2(i, dst, src_ap):
                # spread head-phase loads across both HWDGE queues (SP + ACT);
                # ACT is idle until the first exp
                eng = nc.sync if i % 2 == 0 else nc.scalar
                eng.dma_start(dst, src_ap)

            bq_sb = persist.tile([1, DPC], bf16, name="bq_sb")
            bk_sb = persist.tile([1, DPC], bf16, name="bk_sb")
            bv_sb = persist.tile([1, DPC], bf16, name="bv_sb")
            nc.sync.dma_start(bk_sb[:], bk[None, :])
            nc.scalar.dma_start(bq_sb[:], bq[None, :])
            nc.sync.dma_start(bv_sb[:], bv[None, :])
            ones_row = persist.tile([1, N_CHUNK], bf16, name="ones_row")
            nc.vector.memset(ones_row[:], 1.0)

            wq_sb = persist.tile([P, KO, DPC], bf16, name="wq_sb")
            wk_sb = persist.tile([P, KO, DPC], bf16, name="wk_sb")
            wv_sb = persist.tile([P, KO, DPC], bf16, name="wv_sb")
            wo_sb = persist.tile([P, PO, D], bf16, name="wo_sb")
            for ko in range(KO):
                dma2(
                    ko,
                    wk_sb[:, ko, :],
                    wk.rearrange("(ko p) m -> p ko m", p=P)[:, ko, :],
                )

            # persistent activations
            QT_sb = persist.tile([P, PO, S], bf16, name="QT_sb")   # Q^T[po*128+p, s]
            KT_sb = persist.tile([P, PO, S], bf16, name="KT_sb")
            # V with a ones column per head: PV matmul with lhsT=[V_h | 1]
            # (M=65) yields O^T numerator rows 0..63 and the softmax
            # denominator as row 64 — no separate denominator matmuls.
            V_sb = persist.tile([P, KT, HPC, DK + 1], bf16, name="V_sb")
            nc.vector.memset(V_sb[:, :, :, DK : DK + 1], 1.0)
            OT_sb = persist.tile([P, PO, S], bf16, name="OT_sb")   # O^T (normalized)

            def load_xT_chunk(x_dram, c, tag):
                """[128, KO, 512] bf16 <- x^T[:, c*512:(c+1)*512]
                (split into 4 DMAs so it spreads across queues)"""
                t = xstage.tile([P, KO, N_CHUNK], bf16, tag=tag)
                src = x_dram.rearrange("(ko p) s -> p ko s", p=P)[
                    :, :, c * N_CHUNK : (c + 1) * N_CHUNK
                ]
                if c == 0:
                    for j in range(0, KO, 2):
                        dma2(j // 2, t[:, j : j + 2, :], src[:, j : j + 2, :])
                else:
                    for j in range(0, KO, 2):
                        nc.sync.dma_start(t[:, j : j + 2, :], src[:, j : j + 2, :])
                return t

            _xT_cache = {}

            def get_xT_chunk(x_dram, c, tag):
                key = (tag, c)
                if key not in _xT_cache:
                    _xT_cache[key] = load_xT_chunk(x_dram, c, tag)
                return _xT_cache[key]

            def proj_chain(x_dram, w_sb, b_sb, dst_sb, c, mo, tag):
                """dst_sb[:, mo, c-chunk] = (W^T @ x^T + b)[mo] — one chain."""
                xt = get_xT_chunk(x_dram, c, tag)
                acc = mm512_pool.tile([P, N_CHUNK], f32, tag="mm512")
                for ko in range(KO):
                    nc.tensor.matmul(
                        acc[:],
                        w_sb[:, ko, mo * P : (mo + 1) * P],
                        xt[:, ko, :],
                        start=(ko == 0),
                        stop=(not has_bias and ko == KO - 1),
                    )
                if has_bias:
                    # bias: K=1 outer product  b[m] * ones[s]
                    nc.tensor.matmul(
                        acc[:],
                        b_sb[:, mo * P : (mo + 1) * P],
                        ones_row[:],
                        start=False,
                        stop=True,
                    )
                nc.vector.tensor_copy(
                    dst_sb[:, mo, c * N_CHUNK : (c + 1) * N_CHUNK], acc[:]
                )

            def proj_T(x_dram, w_sb, b_sb, dst_sb, c, tag):
                for mo in range(PO):
                    proj_chain(x_dram, w_sb, b_sb, dst_sb, c, mo, tag)

            _xvT_chunks = {}

            def proj_V_st(st_global):
                """Project V for one 128-row S-tile."""
                c4 = st_global // 4
                if c4 not in _xvT_chunks:
                    _xvT_chunks[c4] = load_xT_chunk(xvT, c4, tag="xvT")
                xt = _xvT_chunks[c4]
                st = st_global % 4
                acc = mm512_pool.tile([P, DPC], f32, tag="mm512")
                for ko in range(KO):
                    nc.tensor.matmul(
                        acc[:],
                        xt[:, ko, st * P : (st + 1) * P],
                        wv_sb[:, ko, :],
                        start=(ko == 0),
                        stop=(not has_bias and ko == KO - 1),
                    )
                if has_bias:
                    # bias: K=1 outer product  ones[s] * b_v[d]
                    nc.tensor.matmul(
                        acc[:],
                        ones_row[:, :P],
                        bv_sb[:],
                        start=False,
                        stop=True,
                    )
                nc.vector.tensor_copy(
                    V_sb[:, st_global, :, :DK],
                    acc[:].rearrange("p (h d) -> p h d", h=HPC),
                )

            def oproj_chunk(c):
                """out[c-rows, :] = O^T.T @ Wo for one S-chunk."""
                for mt in range(4):  # 128-row tiles of S inside chunk
                    row0 = c * N_CHUNK + mt * P
                    for n2 in range(2):  # 512-wide halves of D
                        acc = mm512_pool.tile([P, N_CHUNK], f32, tag="mm512")
                        for po in range(PO):
                            nc.tensor.matmul(
                                acc[:],
                                OT_sb[:, po, row0 : row0 + P],
                                wo_sb[:, po, n2 * N_CHUNK : (n2 + 1) * N_CHUNK],
                                start=(po == 0),
                                stop=(po == PO - 1),
                            )
                        osb = work.tile([P, N_CHUNK], f32, tag="osb")
                        nc.vector.tensor_copy(osb[:], acc[:])
                        nc.sync.dma_start(
                            out[row0 : row0 + P, n2 * N_CHUNK : (n2 + 1) * N_CHUNK],
                            osb[:],
                        )

            # prefetch the first x^T chunks before the remaining weight bulk
            # so the first projection chains start as early as possible
            get_xT_chunk(xkT, 0, "xkT")
            get_xT_chunk(xqT, 0, "xqT")
            for ko in range(KO):
                dma2(
                    ko,
                    wq_sb[:, ko, :],
                    wq.rearrange("(ko p) m -> p ko m", p=P)[:, ko, :],
                )

            # ---- per-chunk: attention (+ first-chunk K/Q/V proj interleaved
            # at kt granularity so exp work starts early), then output proj
            proj_chain(xqT, wq_sb, bq_sb, QT_sb, 0, 0, tag="xqT")
            # V and W_o loads after the first K/Q chains they would delay
            for ko in range(KO):
                nc.sync.dma_start(
                    wv_sb[:, ko, :],
                    wv.rearrange("(ko p) m -> p ko m", p=P)[:, ko, :],
                )
            for po in range(PO):
                nc.sync.dma_start(
                    wo_sb[:, po, :],
                    wo.rearrange("(po p) n -> p po n", p=P)[:, po, :],
                )
            for c in range(N_CHUNKS):
                c_sl = slice(c * N_CHUNK, (c + 1) * N_CHUNK)

                for p in range(PO):  # head pair p = heads (2p, 2p+1)
                    opsums = [
                        psum.tile(
                            [DK + 1, N_CHUNK], f32, tag="opsum", bufs=2,
                            name=f"opsum_{c}_{p}_{i}",
                        )
                        for i in range(2)
                    ]
                    for kt in range(KT):
                        if c == 0 and p == 0:
                            # progressive projections, one chain per kt:
                            # K chain (chunk kt//4, mo kt%4) — mo=0 lands just
                            # in time for this pair; mo 1..3 gate later pairs.
                            # Q chunk-0 chains mo 1..3 early in the sweep.
                            proj_chain(
                                xkT, wk_sb, bk_sb, KT_sb, kt // 4, kt % 4,
                                tag="xkT",
                            )
                            if 1 <= kt <= 3:
                                proj_chain(
                                    xqT, wq_sb, bq_sb, QT_sb, 0, kt, tag="xqT"
                                )
                        spsum = spsum_pool.tile([P, 2, N_CHUNK], f32, tag="spsum")
                        kt_sl = slice(kt * P, (kt + 1) * P)
                        for i in range(2):  # head 2p+i, packed via K=64 row tiling
                            r = slice(64 * i, 64 * (i + 1))
                            nc.tensor.matmul(
                                spsum[:, i, :],
                                KT_sb[r, p, kt_sl],
                                QT_sb[r, p, c_sl],
                            )
                        if with_mask:
                            mb = work.tile([P, N_CHUNK], bf16, tag="mb")
                            nc.sync.dma_start(mb[:], mbias[kt_sl, c_sl])
                            nc.vector.tensor_tensor(
                                spsum[:],
                                spsum[:],
                                mb[:, None, :].to_broadcast((P, 2, N_CHUNK)),
                                mybir.AluOpType.add,
                            )
                        esb = work.tile([P, 2, N_CHUNK], bf16, tag="esb")
                        nc.scalar.activation(
                            esb[:], spsum[:],
                            mybir.ActivationFunctionType.Exp,
                            scale=0.125,
                        )
                        if c == 0 and p == 0:
                            # progressive V projection: S-tile kt right before
                            # its first PV use, spread across the kt loop
                            proj_V_st(kt)
                        first, last = kt == 0, kt == KT - 1
                        for i in range(2):
                            h = 2 * p + i
                            # [O^T numerator; den] = [V_h | 1].T @ E
                            nc.tensor.matmul(
                                opsums[i][:],
                                V_sb[:, kt, h, :],
                                esb[:, i, :],
                                start=first,
                                stop=last,
                            )
                    # normalize: O^T[d, q] /= den[q] (den = row DK of opsum)
                    for i in range(2):
                        # evict PSUM accumulator to SBUF promptly so the next
                        # pair's PV chains get the opsum slot back
                        osum = work.tile([DK + 1, N_CHUNK], f32, tag="osum")
                        nc.vector.tensor_copy(osum[:], opsums[i][:])
                        # reciprocal is ~8 cyc/elem PER LANE: spread the 512
                        # denominators over 64 partitions (DRAM bounce) so it
                        # runs in ~0.1us instead of 3.3us on one lane
                        rd = dram.tile([N_CHUNK], f32, tag="rd")
                        nc.gpsimd.dma_start(rd[None, :], osum[DK : DK + 1, :])
                        dsp = work.tile([DK, N_CHUNK // DK], f32, tag="dsp")
                        nc.gpsimd.dma_start(
                            dsp[:], rd.rearrange("(p e) -> p e", p=DK)
                        )
                        rsp = work.tile([DK, N_CHUNK // DK], f32, tag="rsp")
                        nc.vector.reciprocal(rsp[:], dsp[:])
                        rd2 = dram.tile([N_CHUNK], f32, tag="rd2")
                        nc.gpsimd.dma_start(rd2.rearrange("(p e) -> p e", p=DK), rsp[:])
                        rep = work.tile([DK, N_CHUNK], f32, tag="rep")
                        nc.gpsimd.dma_start(
                            rep[:], rd2[None, :].to_broadcast((DK, N_CHUNK))
                        )
                        if i == 0:
                            # even head: rows 0..63 of this po — write in place
                            nc.vector.tensor_tensor(
                                OT_sb[0:DK, p, c_sl],
                                osum[:DK, :],
                                rep[:],
                                mybir.AluOpType.mult,
                            )
                        else:
                            nt = work.tile([DK, N_CHUNK], bf16, tag="nt")
                            nc.vector.tensor_tensor(
                                nt[:], osum[:DK, :], rep[:], mybir.AluOpType.mult
                            )
                            # place head rows into O^T (DMA shifts partitions)
                            nc.sync.dma_start(OT_sb[DK : 2 * DK, p, c_sl], nt[:])

                # prefetch next chunk's Q^T so its attention starts without a
                # projection stall (runs on PE slack during this chunk)
                if c + 1 < N_CHUNKS:
                    proj_T(xqT, wq_sb, bq_sb, QT_sb, c + 1, tag="xqT")

                # output projection delayed one chunk: emitted here it fills PE
                # slack mid-chunk instead of stalling the next chunk's scores
                if c >= 1:
                    oproj_chunk(c - 1)
                if c == N_CHUNKS - 1:
                    oproj_chunk(c)

    nc.compile()
    return nc


_PROGRAMS: dict = {}


def _get_program(with_mask: bool, has_bias: bool):
    key = (with_mask, has_bias)
    if key not in _PROGRAMS:
        _PROGRAMS[key] = _build_program(with_mask, has_bias)
    return _PROGRAMS[key]


def _shard_inputs(q, k, v, mask, W_q, b_q, W_k, b_k, W_v, b_v, W_o, b_o):
    with_mask = bool((np.asarray(mask) == 0).any())
    has_bias = bool(
        np.asarray(b_q).any() or np.asarray(b_k).any() or np.asarray(b_v).any()
    )
    mb = None
    if with_mask:
        mb = np.where(np.asarray(mask)[0, 0] == 0, np.float32(-30000.0), np.float32(0.0))
        mb = np.ascontiguousarray(mb.T).astype(BF16)

    in_maps = []
    for c in range(8):
        b, hh = c // 2, c % 2
        cols = slice(hh * DPC, (hh + 1) * DPC)
        m = {
            "xqT": np.ascontiguousarray(np.asarray(q)[b].T).astype(BF16),
            "xkT": np.ascontiguousarray(np.asarray(k)[b].T).astype(BF16),
            "xvT": np.ascontiguousarray(np.asarray(v)[b].T).astype(BF16),
            "wq": np.ascontiguousarray(np.asarray(W_q)[:, cols]).astype(BF16),
            "wk": np.ascontiguousarray(np.asarray(W_k)[:, cols]).astype(BF16),
            "wv": np.ascontiguousarray(np.asarray(W_v)[:, cols]).astype(BF16),
            "wo": np.ascontiguousarray(np.asarray(W_o)[cols, :]).astype(BF16),
            "bq": np.ascontiguousarray(np.asarray(b_q)[cols]).astype(BF16),
            "bk": np.ascontiguousarray(np.asarray(b_k)[cols]).astype(BF16),
            "bv": np.ascontiguousarray(np.asarray(b_v)[cols]).astype(BF16),
        }
        if with_mask:
            m["mbias"] = mb
        in_maps.append(m)
    return with_mask, has_bias, in_maps


def kernel(q, k, v, mask, W_q, b_q, W_k, b_k, W_v, b_v, W_o, b_o, **_ignored):
    from concourse.bass_utils import run_bass_kernel_spmd

    with_mask, has_bias, in_maps = _shard_inputs(
        q, k, v, mask, W_q, b_q, W_k, b_k, W_v, b_v, W_o, b_o
    )
    nc = _get_program(with_mask, has_bias)
    res = run_bass_kernel_spmd(nc, in_maps, list(range(8))).results

    bias_final = np.asarray(b_o, np.float32)
    out = np.empty((B, S, D), np.float32)
    for b in range(B):
        out[b] = res[2 * b]["out"] + res[2 * b + 1]["out"] + bias_final
    return out
